# revision 19
# baseline (speedup 1.0000x reference)
"""Longformer-style sparse attention block (nn_BasicNetwork_22892175688067).

Full-input contract: kernel(**inputs) takes the unsharded inputs and returns
the full [B, S, D] fp32 output.  Internally the (batch, head) grid is sharded
across 8 NeuronCores: core = b*4 + hg owns batch b and heads [3*hg, 3*hg+3).
Each core:
  - gathers x[b] = emb[ids[b]] on-device (indirect DMA, bf16 table),
  - transposes x via xbar DMA-transpose (no PE),
  - projects v/q/k (+ global-token projections) for its 3 heads in bf16,
  - runs banded window attention (k-tile-major) + global tokens,
  - applies the output projection slice -> a partial [S, D] fp32 output.
The host sums the 4 per-batch partials (tensor-parallel unshard).
"""

import os
import sys
import types

import numpy as np

for _p in ("/opt/trn_rl_repo",):
    if os.path.isdir(_p) and _p not in sys.path:
        sys.path.insert(0, _p)

import ml_dtypes

BF16 = ml_dtypes.bfloat16

B, S, D = 2, 4096, 768
H, DH, W, G, VOCAB = 12, 64, 256, 16, 50265
HPC = 3               # heads per core
C = S // W            # 16 chunks
QW = W                # queries per chunk (= one-sided window)
P = 128
FT = D // P           # 6 feature k-tiles
TT = S // P           # 32 token tiles
KT = S // P           # 32 key tiles
NCORES = 8
SCALE = 1.0 / 8.0     # 1/sqrt(DH), exact power of two

_COMPILED = {}


def _k_sched():
    """Per k-tile t: (qa, qb, split, segs) where segs = [(c, jt, x0, x1)].

    The q-columns served by k-tile t form one contiguous run [qa, qb).
    split is an aligned piece boundary (relative to qa) <= 512 so each
    psum piece fits one bank and no segment straddles pieces.
    """
    sched = []
    for t in range(KT):
        segs = []
        for c in range(C):
            jt = t - (2 * c - 2)
            if not 0 <= jt <= 5:
                continue
            if c == 0 and jt < 2:
                continue
            if c == C - 1 and jt > 3:
                continue
            x0 = 128 if jt == 5 else 0
            x1 = 128 if jt == 0 else 256
            segs.append((c, jt, x0, x1))
        segs.sort()
        qa = 256 * segs[0][0] + segs[0][2]
        qb = 256 * segs[-1][0] + segs[-1][3]
        # contiguity check
        pos = qa
        for (c, jt, x0, x1) in segs:
            assert 256 * c + x0 == pos
            pos = 256 * c + x1
        assert pos == qb
        # piece split: largest segment-prefix <= 512
        split = 0
        acc = 0
        for (c, jt, x0, x1) in segs:
            if acc + (x1 - x0) <= 512:
                acc += x1 - x0
                split = acc
            else:
                break
        if qb - qa <= 512:
            split = qb - qa
        sched.append((qa, qb, split, segs))
    return sched


SCHED = _k_sched()
FIRST_T = [max(0, 2 * c - 2) for c in range(C)]
LAST_T = [min(KT - 1, 2 * c + 3) for c in range(C)]


def _build_masks():
    j = np.arange(P)[:, None]
    i = np.arange(P)[None, :]
    masks = np.zeros((2, P, P), np.float32)
    masks[0] = (j >= i)      # T_up
    masks[1] = (j <= i)      # T_lo
    return masks.astype(BF16)


def _install_axon_hooks():
    """Provide antenv.axon_hooks (missing in this image) so NTFF tracing works."""
    if "antenv.axon_hooks" in sys.modules:
        return
    mod = types.ModuleType("antenv.axon_hooks")
    hook = [None]
    mod.set_axon_ntff_profile_hook = lambda h: hook.__setitem__(0, h)
    mod.get_axon_ntff_profile_hook = lambda: hook[0]
    sys.modules["antenv.axon_hooks"] = mod
    try:
        import antenv

        antenv.axon_hooks = mod
        from trn_agent_boot.trn_boot import _ntff_profile_via_ctypes

        mod.set_axon_ntff_profile_hook(
            _ntff_profile_via_ctypes("/opt/axon/libaxon_pjrt.so")
        )
    except Exception:
        pass


def _patch_tile_drain():
    """This walrus build allows only ONE sync-wait per instruction.

    Split extra waits onto same-engine NoOps emitted just before the
    instruction (engines execute their stream in order, so chained
    single-wait nops are semantically identical to one multi-wait inst).
    """
    import concourse.mybir as mybir
    import concourse.tile as tile
    from concourse.vector_clock import ScopedClock

    if getattr(tile.TileContext, "_drain_split_patched", False):
        return

    _orig_add = tile.TileContext._add_instruction
    counter = [0]

    def _add_instruction(self, inst):
        si = getattr(inst, "sync_info", None)
        if si is not None and si.on_wait is not None and len(si.on_wait) > 1:
            waits = list(si.on_wait)
            for w in waits[:-1]:
                counter[0] += 1
                nop = mybir.InstNoOp(
                    name=f"WS-{counter[0]}", text_hint="wait_split"
                )
                nop.engine = inst.engine
                nop.sync_info = mybir.SyncInfo(on_wait=[w], on_update=[])
                _orig_add(self, nop)
            inst.sync_info = mybir.SyncInfo(
                on_wait=[waits[-1]], on_update=list(si.on_update)
            )
        _orig_add(self, inst)

    tile.TileContext._add_instruction = _add_instruction

    def _drain_and_barrier(self, tick_clock, wait_clock):
        drain1 = self.nc.sync.drain()
        wait_clock.add_sem_waits(
            drain1.ins, ScopedClock({None: tick_clock.global_clock})
        )
        si = drain1.ins.sync_info
        waits = list(si.on_wait) if si is not None and si.on_wait else []
        if len(waits) > 1:
            drain1.ins.sync_info = mybir.SyncInfo(
                on_wait=waits[:1], on_update=list(si.on_update)
            )
            for wchunk in waits[1:]:
                d = self.nc.sync.drain()
                d.ins.sync_info = mybir.SyncInfo(on_wait=[wchunk], on_update=[])
        self.nc.all_engine_barrier()
        assert self.sems is not None
        popped = self.nc._tile_sem_poison_stack.pop()
        assert popped is self._sem_poison
        self.nc.clear_and_free_semaphores(list(self.sems.allocated().values()))
        self.nc.all_engine_barrier()

    tile.TileContext._drain_and_barrier = _drain_and_barrier
    tile.TileContext._drain_split_patched = True


def build_nc():
    """Build the per-core Bass program (identical on all 8 cores)."""
    import concourse.bass as bass
    import concourse.mybir as mybir
    import concourse.tile as tile

    _patch_tile_drain()

    f32 = mybir.dt.float32
    bf16 = mybir.dt.bfloat16
    i32 = mybir.dt.int32
    AF = mybir.ActivationFunctionType
    OP = mybir.AluOpType

    nc = bass.Bass("TRN2", num_devices=NCORES)

    ids = nc.dram_tensor("ids", [S, 1], i32, kind="ExternalInput")
    emb = nc.dram_tensor("emb", [VOCAB, D], bf16, kind="ExternalInput")
    wqk = nc.dram_tensor("wqk", [D, 768], bf16, kind="ExternalInput")
    bqk = nc.dram_tensor("bqk", [768], f32, kind="ExternalInput")
    wv = nc.dram_tensor("wv", [D, 384], bf16, kind="ExternalInput")
    bv = nc.dram_tensor("bv", [384], f32, kind="ExternalInput")
    wo0 = nc.dram_tensor("wo0", [P, D], bf16, kind="ExternalInput")
    wo1 = nc.dram_tensor("wo1", [DH + 1, D], bf16, kind="ExternalInput")
    msk = nc.dram_tensor("msk", [2, P, P], bf16, kind="ExternalInput")
    outp = nc.dram_tensor("out", [S, D], f32, kind="ExternalOutput")
    DEBUG = bool(int(os.environ.get("KERNEL_DEBUG", "0")))
    if DEBUG:
        dbg_ctx = nc.dram_tensor("dbg_ctx", [3, DH, S], bf16, kind="ExternalOutput")
        dbg_pg = nc.dram_tensor("dbg_pg", [P, TT, HPC * G], bf16, kind="ExternalOutput")
        dbg_xt = nc.dram_tensor("dbg_xt", [P, FT, S], bf16, kind="ExternalOutput")

    with tile.TileContext(nc) as tc:
        from contextlib import ExitStack

        with ExitStack() as ctx:
            pers = ctx.enter_context(tc.tile_pool(name="pers", bufs=1))

            # ---------------- persistent SBUF tensors ----------------
            wqk_sb = pers.tile([P, FT, 768], bf16, tag="wqk")
            bqk_sb = pers.tile([P, FT], f32, tag="bqk")
            wv_sb = pers.tile([P, FT, 384], bf16, tag="wv")
            wo0_sb = pers.tile([P, D], bf16, tag="wo0")
            wo1_sb = pers.tile([DH + 1, D], bf16, tag="wo1")
            msk_sb = pers.tile([P, 2, P], bf16, tag="msk")
            ones_sb = pers.tile([P, P], f32, tag="ones")
            onesb_sb = pers.tile([1, DH], bf16, tag="onesb")
            bv1_sb = pers.tile([1, 384], f32, tag="bv1")
            bvb_sb = pers.tile([P, 384], f32, tag="bvb")
            ids_sb = pers.tile([P, TT], i32, tag="idsb")
            # x transposed (feature-major), projection outputs
            xT_sb = pers.tile([P, FT, S], bf16, tag="xT")
            blk_sb = pers.tile([P, 5, S], bf16, tag="blk")
            qg5_sb = pers.tile([P, P], bf16, tag="qg5")
            v_sb = pers.tile([P, TT, 2 * HPC, DH + 1], bf16, tag="vsb")
            ctx01_sb = pers.tile([P, S], bf16, tag="ctx01")
            ctx2e_sb = pers.tile([DH + 1, S], bf16, tag="ctx2e")
            ctxh1_sb = pers.tile([DH, S], bf16, tag="ctxh1")
            pg_sb = pers.tile([P, TT, HPC * G], bf16, tag="pgsb")
            ident_sb = pers.tile([G, G], bf16, tag="ident")

            # ---------------- constant loads ----------------
            nc.sync.dma_start(
                out=wqk_sb[:], in_=wqk.ap().rearrange("(kt p) c -> p kt c", p=P)
            )
            nc.sync.dma_start(
                out=bqk_sb[:], in_=bqk.ap().rearrange("(kt p) -> p kt", p=P)
            )
            nc.sync.dma_start(
                out=wv_sb[:], in_=wv.ap().rearrange("(kt p) c -> p kt c", p=P)
            )
            nc.sync.dma_start(out=wo0_sb[:], in_=wo0.ap())
            nc.sync.dma_start(out=wo1_sb[:], in_=wo1.ap())
            nc.sync.dma_start(out=msk_sb[:], in_=msk.ap().rearrange("m p i -> p m i"))
            nc.sync.dma_start(out=bv1_sb[:], in_=bv.ap()[None, :])
            nc.sync.dma_start(
                out=ids_sb[:], in_=ids.ap().rearrange("(t p) o -> p (t o)", p=P)
            )
            nc.gpsimd.memset(ones_sb[:], 1.0)
            nc.gpsimd.memset(onesb_sb[:], 1.0)
            nc.gpsimd.memset(v_sb[:, :, :, DH], 1.0)
            nc.gpsimd.memset(ctx2e_sb[DH : DH + 1, :], 1.0)
            from concourse.masks import make_identity

            make_identity(nc, ident_sb[:])

            # broadcast v-bias across partitions via ones-matmul (fp32, once)
            with tc.tile_pool(name="bcast_ps", bufs=1, space="PSUM") as bps:
                bvp = bps.tile([P, 384], f32, tag="bvp")
                nc.tensor.matmul(
                    out=bvp[:], lhsT=ones_sb[0:1, :], rhs=bv1_sb[:],
                    start=True, stop=True,
                )
                nc.vector.tensor_copy(out=bvb_sb[:], in_=bvp[:])

            # ---------------- gather + xbar transpose (DMA only) ------------
            with tc.tile_pool(name="gather", bufs=4) as gp:
                for tt in range(TT):
                    xg = gp.tile([P, D], bf16, tag="xg")
                    nc.gpsimd.indirect_dma_start(
                        out=xg[:],
                        out_offset=None,
                        in_=emb.ap(),
                        in_offset=bass.IndirectOffsetOnAxis(
                            ap=ids_sb[:, tt : tt + 1], axis=0
                        ),
                    )
                    nc.sync.dma_start(
                        out=xT_sb[:, :, tt * P : (tt + 1) * P],
                        in_=xg[:],
                        transpose=True,
                    )

            # ---------------- projections ----------------
            # v/vg first (token-major, per-tile -> PE busy while gathers run)
            with tc.tile_pool(name="vp_ps", bufs=3, space="PSUM") as vpp:
                for tt in range(TT):
                    vp = vpp.tile([P, 384], f32, tag="vps")
                    for kt in range(FT):
                        nc.tensor.matmul(
                            out=vp[:],
                            lhsT=xT_sb[:, kt, tt * P : (tt + 1) * P],
                            rhs=wv_sb[:, kt, :],
                            start=(kt == 0),
                            stop=(kt == FT - 1),
                        )
                    nc.vector.tensor_tensor(
                        out=v_sb[:, tt, :, 0:DH],
                        in0=vp[:],
                        in1=bvb_sb[:],
                        op=OP.add,
                    )
                # qg block (B5) over token-tile 0 only
                ps5 = vpp.tile([P, 384], f32, tag="vps")
                for kt in range(FT):
                    nc.tensor.matmul(
                        out=ps5[:, 0:P],
                        lhsT=wqk_sb[:, kt, 5 * P : 6 * P],
                        rhs=xT_sb[:, kt, 0:P],
                        start=(kt == 0),
                        stop=(kt == FT - 1),
                    )
                nc.vector.tensor_scalar_add(
                    out=qg5_sb[:], in0=ps5[:, 0:P], scalar1=bqk_sb[:, 5:6]
                )

            # q/k/qg/kg projections (feature-major outputs).
            # block cols: 0:(q0|q1) 1:(k0|k1) 2:(q2|qg2) 3:(k2|kg2) 4:(kg0|kg1)
            with tc.tile_pool(name="proj_ps", bufs=3, space="PSUM") as pps:
                for n in range(S // 512):
                    for bi in range(5):
                        ps = pps.tile([P, 512], f32, tag="pps")
                        for kt in range(FT):
                            nc.tensor.matmul(
                                out=ps[:],
                                lhsT=wqk_sb[:, kt, bi * P : (bi + 1) * P],
                                rhs=xT_sb[:, kt, n * 512 : (n + 1) * 512],
                                start=(kt == 0),
                                stop=(kt == FT - 1),
                            )
                        nc.vector.tensor_scalar_add(
                            out=blk_sb[:, bi, n * 512 : (n + 1) * 512],
                            in0=ps[:],
                            scalar1=bqk_sb[:, bi : bi + 1],
                        )

            # operand views (each matmul operand pair shares a base partition)
            qv = [blk_sb[0:DH, 0, :], blk_sb[DH:P, 0, :], blk_sb[0:DH, 2, :]]
            kv = [blk_sb[0:DH, 1, :], blk_sb[DH:P, 1, :], blk_sb[0:DH, 3, :]]
            qgv = [qg5_sb[0:DH, 0:G], qg5_sb[DH:P, 0:G], blk_sb[DH:P, 2, 0:G]]
            kgv = [blk_sb[0:DH, 4, :], blk_sb[DH:P, 4, :], blk_sb[DH:P, 3, :]]
            ctxdst = [ctx01_sb[0:DH, :], ctxh1_sb[:, :], ctx2e_sb[0:DH, :]]

            # ---------------- banded window attention (k-tile-major) --------
            with tc.tile_pool(name="att_p", bufs=4) as app, \
                 tc.tile_pool(name="att_s", bufs=4) as asb, \
                 tc.tile_pool(name="sp_ps", bufs=4, space="PSUM") as sps, \
                 tc.tile_pool(name="ctx_ps", bufs=3, space="PSUM") as cps, \
                 tc.tile_pool(name="bc_ps", bufs=1, space="PSUM") as bps2:

                for h in range(HPC):
                    pg_t = {}       # c -> pg tile
                    ctx_t = {}      # c -> (pair tile, col0)
                    started = set() # chunks whose PV accumulation has begun
                    prev = None     # (t, pieces) of previous iteration

                    for t in range(KT + 1):
                        if t < KT:
                            qa, qb, split, segs = SCHED[t]
                            # --- scores: 1-2 contiguous matmul pieces ---
                            pieces = []
                            for (p0, p1) in ((0, split), (split, qb - qa)):
                                w = p1 - p0
                                if w <= 0:
                                    continue
                                sp = sps.tile([P, 512], f32, tag="sp")
                                nc.tensor.matmul(
                                    out=sp[:, 0:w],
                                    lhsT=kv[h][:, t * P : (t + 1) * P],
                                    rhs=qv[h][:, qa + p0 : qa + p1],
                                    start=True,
                                    stop=True,
                                )
                                pt = app.tile([P, 512], bf16, tag="p")
                                nc.scalar.activation(pt[:, 0:w], sp[:, 0:w], AF.Exp)
                                pieces.append((p0, p1, pt))
                            # --- masks (in-place on p) ---
                            for (c, jt, x0, x1) in segs:
                                if jt in (0, 1, 4, 5):
                                    # triangle position within the run
                                    if jt == 0:
                                        m0 = 256 * c + 0 - qa
                                        mid = 0
                                    elif jt == 1:
                                        m0 = 256 * c + 128 - qa
                                        mid = 0
                                    elif jt == 4:
                                        m0 = 256 * c + 0 - qa
                                        mid = 1
                                    else:
                                        m0 = 256 * c + 128 - qa
                                        mid = 1
                                    for (p0, p1, pt) in pieces:
                                        if p0 <= m0 < p1:
                                            o = m0 - p0
                                            eng = nc.vector if (t + c) % 2 else nc.gpsimd
                                            eng.tensor_tensor(
                                                out=pt[:, o : o + P],
                                                in0=pt[:, o : o + P],
                                                in1=msk_sb[:, mid, :],
                                                op=OP.mult,
                                            )
                            if t == 0:
                                # global keys (0:16) excluded from the window
                                for (p0, p1, pt) in pieces:
                                    nc.gpsimd.memset(pt[0:G, 0 : p1 - p0], 0.0)
                            # --- global-key scores for newly-appearing chunks
                            newc = [0, 1] if t == 0 else (
                                [t // 2 + 1] if (t % 2 == 0 and t // 2 + 1 < C)
                                else []
                            )
                            for c in newc:
                                sg = sps.tile([P, 512], f32, tag="sp")
                                nc.tensor.matmul(
                                    out=sg[0:G, 0:256],
                                    lhsT=kv[h][:, 0:G],
                                    rhs=qv[h][:, c * QW : (c + 1) * QW],
                                    start=True,
                                    stop=True,
                                )
                                pgt = app.tile([G, 256], bf16, tag="pg")
                                nc.scalar.activation(pgt[:], sg[0:G, 0:256], AF.Exp)
                                pg_t[c] = pgt

                        # --- PV for previous k-tile ---
                        if prev is not None:
                            tp, ppieces = prev
                            qa_p, qb_p, _, segs_p = SCHED[tp]
                            for (c, jt, x0, x1) in segs_p:
                                if c not in started:
                                    # first contribution: allocate acc & pg-PV
                                    cpt = cps.tile(
                                        [DH + 1, 256], f32, tag="cp", name="cpt"
                                    )
                                    ctx_t[c] = cpt
                                    started.add(c)
                                    nc.tensor.matmul(
                                        out=ctx_t[c][:],
                                        lhsT=v_sb[0:G, 0, h, :],
                                        rhs=pg_t[c][:],
                                        start=True,
                                        stop=False,
                                    )
                                pair = ctx_t[c]
                                # locate the p piece holding this segment
                                s0 = 256 * c + x0 - qa_p
                                s1 = 256 * c + x1 - qa_p
                                for (p0, p1, pt) in ppieces:
                                    if p0 <= s0 < p1:
                                        nc.tensor.matmul(
                                            out=pair[:, x0:x1],
                                            lhsT=v_sb[:, tp, h, :],
                                            rhs=pt[:, s0 - p0 : s1 - p0],
                                            start=False,
                                            stop=(tp == LAST_T[c]),
                                        )
                                # --- finalize chunk when complete ---
                                if tp == LAST_T[c]:
                                    rc = asb.tile([1, 256], f32, tag="rc")
                                    nc.vector.reciprocal(
                                        out=rc[:], in_=pair[DH : DH + 1, :]
                                    )
                                    rcb = asb.tile([1, 256], bf16, tag="rcb")
                                    nc.vector.tensor_copy(out=rcb[:], in_=rc[:])
                                    bc = bps2.tile([DH, 256], f32, tag="bc")
                                    nc.tensor.matmul(
                                        out=bc[:],
                                        lhsT=onesb_sb[:],
                                        rhs=rcb[:],
                                        start=True,
                                        stop=True,
                                    )
                                    bcs = asb.tile([DH, 256], f32, tag="bcs")
                                    nc.vector.tensor_copy(out=bcs[:], in_=bc[:])
                                    nc.vector.tensor_tensor(
                                        out=ctxdst[h][:, c * QW : (c + 1) * QW],
                                        in0=pair[0:DH, :],
                                        in1=bcs[:],
                                        op=OP.mult,
                                    )
                        if t < KT:
                            prev = (t, pieces)

            # ---------------- global query rows ----------------
            with tc.tile_pool(name="g_s", bufs=4) as asb, \
                 tc.tile_pool(name="g_sp", bufs=3, space="PSUM") as sps, \
                 tc.tile_pool(name="g_ps", bufs=2, space="PSUM") as bps2:
                for h in range(HPC):
                    # scores vs all tokens (token-major out), exp
                    for tb in range(TT // 4):
                        gp_ps = sps.tile([P, 512], f32, tag="sp")
                        for k in range(4):
                            tt = tb * 4 + k
                            nc.tensor.matmul(
                                out=gp_ps[:, k * G : (k + 1) * G],
                                lhsT=kgv[h][:, tt * P : (tt + 1) * P],
                                rhs=qgv[h][:],
                                start=True,
                                stop=True,
                            )
                        nc.scalar.activation(
                            pg_sb[:, tb * 4 : (tb + 1) * 4, h * G : (h + 1) * G],
                            gp_ps[:, 0 : 4 * G],
                            AF.Exp,
                        )
                    # PV with p as stationary -> g-major ctx [16, 65]
                    gps = bps2.tile([G, DH + 1], f32, tag="gps")
                    for tt in range(TT):
                        nc.tensor.matmul(
                            out=gps[:],
                            lhsT=pg_sb[:, tt, h * G : (h + 1) * G],
                            rhs=v_sb[:, tt, HPC + h, :],
                            start=(tt == 0),
                            stop=(tt == TT - 1),
                        )
                    rg = asb.tile([G, 1], f32, tag="rg")
                    nc.vector.reciprocal(out=rg[:], in_=gps[:, DH : DH + 1])
                    gn = asb.tile([G, DH], bf16, tag="gn")
                    nc.vector.tensor_scalar_mul(
                        out=gn[:], in0=gps[:, 0:DH], scalar1=rg[:]
                    )
                    gt = bps2.tile([DH, G], bf16, tag="gt")
                    nc.tensor.transpose(out=gt[:], in_=gn[:], identity=ident_sb[:])
                    nc.vector.tensor_copy(out=ctxdst[h][:, 0:G], in_=gt[:])

                # head 1 ctx lives at base partition 0; move to rows 64:128
                nc.sync.dma_start(out=ctx01_sb[DH:P, :], in_=ctxh1_sb[:])

            if DEBUG:
                nc.sync.dma_start(out=dbg_ctx.ap()[0], in_=ctx01_sb[0:DH, :])
                nc.sync.dma_start(out=dbg_ctx.ap()[1], in_=ctx01_sb[DH:P, :])
                nc.sync.dma_start(out=dbg_ctx.ap()[2], in_=ctx2e_sb[0:DH, :])
                nc.sync.dma_start(out=dbg_pg.ap(), in_=pg_sb[:])
                nc.sync.dma_start(out=dbg_xt.ap(), in_=xT_sb[:])

            # ---------------- output projection ----------------
            with tc.tile_pool(name="out_sb", bufs=3) as osb, \
                 tc.tile_pool(name="out_ps", bufs=2, space="PSUM") as ops:
                for tt in range(TT):
                    op_ps = ops.tile([P, D], f32, tag="ops")
                    for (n0, n1) in ((0, 512), (512, 768)):
                        nc.tensor.matmul(
                            out=op_ps[:, n0:n1],
                            lhsT=ctx01_sb[:, tt * P : (tt + 1) * P],
                            rhs=wo0_sb[:, n0:n1],
                            start=True,
                            stop=False,
                        )
                        nc.tensor.matmul(
                            out=op_ps[:, n0:n1],
                            lhsT=ctx2e_sb[:, tt * P : (tt + 1) * P],
                            rhs=wo1_sb[:, n0:n1],
                            start=False,
                            stop=True,
                        )
                    ot = osb.tile([P, D], f32, tag="ot")
                    nc.scalar.copy(out=ot[:], in_=op_ps[:])
                    nc.sync.dma_start(
                        out=outp.ap()[tt * P : (tt + 1) * P, :], in_=ot[:]
                    )

    return nc


def _prep_core_inputs(core, input_ids, emb, Wq, bq, Wk, bk, Wv, bv,
                      Wqg, bqg, Wkg, bkg, Wvg, bvg, Wo, bo):
    b, hg = divmod(core, 4)
    hs = HPC * hg * DH           # feature offset of this core's head slice
    sl = slice(hs, hs + HPC * DH)

    def hcol(Wm, h):
        return np.asarray(Wm[:, hs + h * DH : hs + (h + 1) * DH], np.float32)

    def hbias(bm, h):
        return np.asarray(bm[hs + h * DH : hs + (h + 1) * DH], np.float32)

    # blocks: 0:(q0|q1) 1:(k0|k1) 2:(q2|qg2) 3:(k2|kg2) 4:(kg0|kg1) 5:(qg0|qg1)
    wq = [hcol(Wq, h) * SCALE for h in range(HPC)]
    wk = [hcol(Wk, h) for h in range(HPC)]
    wqg = [hcol(Wqg, h) * SCALE for h in range(HPC)]
    wkg = [hcol(Wkg, h) for h in range(HPC)]
    bq_ = [hbias(bq, h) * SCALE for h in range(HPC)]
    bk_ = [hbias(bk, h) for h in range(HPC)]
    bqg_ = [hbias(bqg, h) * SCALE for h in range(HPC)]
    bkg_ = [hbias(bkg, h) for h in range(HPC)]

    wqk_cat = np.concatenate(
        [wq[0], wq[1], wk[0], wk[1], wq[2], wqg[2], wk[2], wkg[2],
         wkg[0], wkg[1], wqg[0], wqg[1]], axis=1)
    bqk_cat = np.concatenate(
        [bq_[0], bq_[1], bk_[0], bk_[1], bq_[2], bqg_[2], bk_[2], bkg_[2],
         bkg_[0], bkg_[1], bqg_[0], bqg_[1]])

    wv_cat = np.concatenate(
        [hcol(Wv, h) for h in range(HPC)] + [hcol(Wvg, h) for h in range(HPC)],
        axis=1)
    bv_cat = np.concatenate(
        [hbias(bv, h) for h in range(HPC)] + [hbias(bvg, h) for h in range(HPC)])

    wo_cat = np.asarray(Wo[sl, :], np.float32)
    bo_in = np.asarray(bo, np.float32) if hg == 0 else np.zeros(
        (D,), np.float32)
    wo1_ext = np.concatenate(
        [wo_cat[P : P + DH, :], bo_in[None, :]], axis=0)

    global _EMB_BF16
    if "_EMB_BF16" not in globals() or _EMB_BF16[0] is not emb:
        _EMB_BF16 = (emb, np.ascontiguousarray(np.asarray(emb)).astype(BF16))

    return {
        "ids": np.asarray(input_ids[b], np.int32).reshape(S, 1),
        "emb": _EMB_BF16[1],
        "wqk": wqk_cat.astype(BF16),
        "bqk": bqk_cat.astype(np.float32),
        "wv": wv_cat.astype(BF16),
        "bv": bv_cat.astype(np.float32),
        "wo0": np.ascontiguousarray(wo_cat[0:P, :]).astype(BF16),
        "wo1": np.ascontiguousarray(wo1_ext).astype(BF16),
        "msk": _build_masks(),
    }


def kernel(**inputs):
    _install_axon_hooks()
    from concourse.bass_utils import run_bass_kernel_spmd

    if "nc" not in _COMPILED:
        _COMPILED["nc"] = build_nc()
    nc = _COMPILED["nc"]

    in_maps = [_prep_core_inputs(core, **inputs) for core in range(NCORES)]
    trace = bool(int(os.environ.get("KERNEL_TRACE", "0")))
    res = run_bass_kernel_spmd(nc, in_maps, list(range(NCORES)), trace=trace)
    _COMPILED["last_result"] = res

    out = np.zeros((B, S, D), np.float32)
    for core in range(NCORES):
        out[core // 4] += res.results[core]["out"]
    return out
